# revision 35
# baseline (speedup 1.0000x reference)
"""MoE (B=2,T=2048,D=768,E=8,K=2,H=1536) Trainium2 kernel.

Sparse expert-parallel over the 8 NeuronCores: the host computes the gate
(softmax + top-2) in numpy, gathers the tokens routed to each expert, and
core e runs expert e's FFN only on its ~B*T*K/E gathered tokens. The
per-token gate weight is applied on device; the host scatter-adds the two
weighted expert outputs per token.

Activations stay feature-major (x^T [D, tok]) so gate/up banks [D,H] and
the down bank [H,D] are already in the stationary-operand (lhsT) layout the
PE wants — no transposes on device. The big GEMMs run in bf16 (full PE
rate, automatic fast-weight-load so LDWEIGHTS hides under the matmuls,
half the HBM traffic of f32; accumulation stays f32 in PSUM). The token
count is padded only to a multiple of 8 — tokens are the free (moving)
dim, so no 128-granularity is needed.
"""

import numpy as np
import ml_dtypes

import concourse.bass as bass
import concourse.mybir as mybir
import concourse.tile as tile
from concourse import bass_utils

# Problem shape (hardcoded per contract).
B, T, D, E, H, KTOP = 2, 2048, 768, 8, 1536, 2
NTOK = B * T            # 4096 tokens
DC = D // 128           # 6 chunks of the D (contraction) dim
HC = H // 128           # 12 chunks of the H dim
F32 = mybir.dt.float32
F32R = mybir.dt.float32r
BF16 = mybir.dt.bfloat16
NPBF16 = ml_dtypes.bfloat16
N_WARMUP = 12           # HAM warmup matmuls (8x N=512 + 4x N=256)


def _blocks(npad):
    """Token blocks, each a multiple of 8 and <=512 (PSUM bank limit).
    The first block is kept small so its x DMA lands early and the PE
    starts sooner; the rest are balanced."""
    if npad <= 512:
        return [(0, npad)]
    first = min(384, npad // 4 * 4)
    rest = npad - first
    nblk = max(1, -(-rest // 512))
    base = rest // nblk // 4 * 4
    sizes = [first] + [base] * nblk
    rem = rest - base * nblk
    i = 1
    while rem > 0:
        take = min(4, rem)
        sizes[i] += take
        rem -= take
        i = 1 + (i % nblk)
    blocks = []
    off = 0
    for s in sizes:
        blocks.append((off, s))
        off += s
    return blocks


def _install_axon_ntff_hook():
    """Best-effort: register the antenv.axon_hooks NTFF profile hook that the
    agent image lacks, so trace=True (or BASS_TRACE=1) can profile under axon.
    Never raises."""
    try:
        import sys, types, contextlib, ctypes  # noqa: PLC0415
        import antenv  # noqa: PLC0415
        if "antenv.axon_hooks" in sys.modules:
            return
        _HOOK = [None]
        mod = types.ModuleType("antenv.axon_hooks")
        mod.set_axon_ntff_profile_hook = lambda h: _HOOK.__setitem__(0, h)
        mod.get_axon_ntff_profile_hook = lambda: _HOOK[0]
        sys.modules["antenv.axon_hooks"] = mod
        antenv.axon_hooks = mod

        lib = ctypes.CDLL("/opt/axon/libaxon_pjrt.so")
        if not hasattr(lib, "axon_start_nrt_profile"):
            return
        lib.axon_start_nrt_profile.argtypes = [
            ctypes.POINTER(ctypes.c_int64), ctypes.c_size_t]
        lib.axon_start_nrt_profile.restype = ctypes.c_int64
        lib.axon_stop_nrt_profile.argtypes = [ctypes.c_char_p]
        lib.axon_stop_nrt_profile.restype = ctypes.c_int64

        @contextlib.contextmanager
        def _hook(output_dir, device_ids):
            import jax  # noqa: PLC0415
            jax.devices()
            if device_ids:
                ids = (ctypes.c_int64 * len(device_ids))(*device_ids)
                rc = lib.axon_start_nrt_profile(ids, len(device_ids))
            else:
                rc = lib.axon_start_nrt_profile(None, 0)
            if rc != 0:
                raise RuntimeError(f"axon_start_nrt_profile rc={rc}")
            try:
                yield
            finally:
                lib.axon_stop_nrt_profile(str(output_dir).encode())

        mod.set_axon_ntff_profile_hook(_hook)
    except Exception:
        pass


def _split_multiwaits(nc):
    """This walrus build only supports one sync-wait per instruction; move
    extra waits onto preceding NOPs on the same engine."""
    for fn in nc.m.functions:
        for bb in fn.blocks:
            out = []
            for ins in bb.instructions:
                si = ins.sync_info
                if si is not None and si.on_wait is not None and len(si.on_wait) > 1:
                    waits = list(si.on_wait)
                    for i, w in enumerate(waits[:-1]):
                        out.append(mybir.InstNoOp(
                            name=f"{ins.name}-sw{i}",
                            engine=ins.engine,
                            sync_info=mybir.SyncInfo(on_wait=[w], on_update=[]),
                        ))
                    si.on_wait = [waits[-1]]
                    ins.sync_info = si
                out.append(ins)
            bb.instructions = out
    return nc


def _hoist_first_dmas(nc, k=2):
    """Move the first `k` wait-free SP DMA dispatches above the TileContext
    entry barrier so their descriptors are written (and transfers run) while
    the other engines are still in the barrier. Saves ~1us of time-to-first-
    matmul; the hoisted DMAs have no waits and nothing before the barrier
    touches their target tiles."""
    fn = nc.m.functions[0]

    def hoist(engine, types, kk, contiguous=False):
        moved = []
        for bb in fn.blocks:
            for ins in bb.instructions:
                if ins.engine != engine:
                    continue
                if isinstance(ins, types):
                    si = ins.sync_info
                    if contiguous or si is None or not si.on_wait:
                        moved.append((bb, ins))
                    if len(moved) >= kk:
                        break
                elif contiguous and moved:
                    break
            if moved:
                break
        if not moved:
            return
        bb0 = fn.blocks[0]
        idx = None
        for i, ins in enumerate(bb0.instructions):
            if ins.engine == engine and isinstance(ins, mybir.InstDrain):
                idx = i
                break
        if idx is None:
            return
        for bb, ins in moved:
            bb.instructions.remove(ins)
        bb0.instructions[idx:idx] = [ins for _, ins in moved]

    hoist(mybir.EngineType.SP, mybir.InstDMACopy, k)
    # The HAM-warmup memset too: it gates the warmup matmuls, and the DMA
    # hoist above delays every engine's body start by the extra dispatch
    # time.
    hoist(mybir.EngineType.DVE, mybir.InstMemset, 1)
    # And the warmup ldweights/matmuls themselves (the contiguous run at
    # the PE body head, incl. the memset-sem wait on the first): with them
    # above the PE barrier, the PE array warms to 2.4 GHz during the
    # barrier window instead of idling in it, so the real MM stream starts
    # fully warm.
    hoist(mybir.EngineType.PE, (mybir.InstLdweights, mybir.InstMatmult),
          2 * N_WARMUP, contiguous=True)
    return nc


def build_nc(npad):
    """Expert FFN on `npad` gathered tokens (feature-major, bf16 GEMMs)."""
    blocks = _blocks(npad)
    nblk = len(blocks)

    # All inputs are host-pre-swizzled to the exact SBUF layout so every DMA
    # is fully contiguous per partition (multi-KB lines -> peak DMA BW):
    #   xg_s[p, off*DC + c*tb + t] = x^T[c*128+p, off+t]       (block-major)
    #   gb_s[p, (ht*DC + c)*128 + j] = gate_bank[c*128+p, ht*128+j]
    #   db_s[p, (hk*DC + c)*128 + j] = down_bank[hk*128+p, c*128+j]
    # Output y_s uses the same block-major layout as xg_s.
    nc = bass.Bass()
    xg_s = nc.dram_tensor("xg_s", [128, DC * npad], BF16, kind="ExternalInput")
    gb_s = nc.dram_tensor("gb_s", [128, DC * H], BF16, kind="ExternalInput")
    ub_s = nc.dram_tensor("ub_s", [128, DC * H], BF16, kind="ExternalInput")
    db_s = nc.dram_tensor("db_s", [128, HC * D], BF16, kind="ExternalInput")
    wrow = nc.dram_tensor("wrow", [1, npad], F32R, kind="ExternalInput")
    onesd = nc.dram_tensor("onesd", [1, 128], F32R, kind="ExternalInput")
    y_s = nc.dram_tensor("y_s", [128, DC * npad], BF16, kind="ExternalOutput")

    with tile.TileContext(nc) as tc:
        with (
            tc.tile_pool(name="wts", bufs=1) as wts,
            tc.tile_pool(name="xp", bufs=1) as xp,
            tc.tile_pool(name="hp", bufs=24) as hp,
            tc.tile_pool(name="sap", bufs=2) as sap,
            tc.tile_pool(name="yp", bufs=3) as yp,
            tc.tile_pool(name="wsp", bufs=2) as wsp,
            tc.tile_pool(name="ps", bufs=8, space="PSUM") as ps,
        ):
            # DMA plan. Everything rides the SP HWDGE ring (strict FIFO,
            # low latency) in EXACT consumption order, so each chunk lands
            # just ahead of the matmuls that need it and the full 358 GB/s
            # serves the critical chunk at every moment. The gate/up banks
            # stream as interleaved ht-chunks (1-ht chunks for ht0/ht1 to
            # minimize time-to-first-matmul, ht-pairs after); x / down-bank
            # / gate-weight chunks are slotted at their need times. No
            # SWDGE: its ~2us fixed latency and unpaced concurrency made
            # chunks land late (observed 2.5us PE stalls).
            HTW = DC * 128                     # swizzled width of one ht slice
            NH2 = HC // 2
            CHUNKS = [(0, 1), (1, 1), (2, 2), (4, 2), (6, 2), (8, 2), (10, 2)]
            gbC = [None if i == 0 else
                   wts.tile([128, n, DC, 128], BF16, tag=f"gbC{i}",
                            name=f"gbC{i}") for i, (s, n) in enumerate(CHUNKS)]
            ubC = [wts.tile([128, n, DC, 128], BF16, tag=f"ubC{i}",
                            name=f"ubC{i}") for i, (s, n) in enumerate(CHUNKS)]
            # ht0's gate bank is further split (k0 alone, then k1-5) so the
            # very first LDWEIGHTS+matmul fire as soon as a 32KB sliver has
            # landed instead of after the whole ht0 slice.
            gb0a = wts.tile([128, 128], BF16, name="gb0a")
            gb0b = wts.tile([128, DC - 1, 128], BF16, name="gb0b")
            db_h = [wts.tile([128, NH2, DC, 128], BF16, tag=f"dbh{i}",
                             name=f"dbh{i}") for i in range(2)]
            ones_sb = wts.tile([1, 128], F32R)
            wrow_sb = wts.tile([1, npad], F32R)

            xbs = [None] * nblk

            def emit_xb_dma(bi, ring):
                off, tb = blocks[bi]
                xbt = xp.tile([128, DC, tb], BF16,
                              tag=f"xb{bi}", name=f"xb{bi}")
                ins = ring.dma_start(xbt[:], xg_s[:, off * DC:(off + tb) * DC])
                xbs[bi] = xbt
                return ins

            # Block 0's x is split into two tiles (k 0-2, k 3-5) so the ht0
            # matmuls can begin on the first half while the second is still
            # in flight.
            XSPL = DC // 2
            tb0 = blocks[0][1]
            x0a = wts.tile([128, XSPL, tb0], BF16, name="x0a")
            x0b = wts.tile([128, DC - XSPL, tb0], BF16, name="x0b")

            def xslice(bi, k):
                if bi == 0:
                    if k < XSPL:
                        return x0a[:, k, :]
                    return x0b[:, k - XSPL, :]
                return xbs[bi][:, k, :]

            # HAM warmup: the PE clock-gate defaults to 1.2 GHz and only
            # reaches 2.4 GHz after ~3.4us of sustained activity. Feed it
            # throwaway matmuls on a memset scratch tile (no DMA dependency,
            # so they start right after the framework preamble) so the real
            # MM stream runs warm from its first instruction.
            wu = wts.tile([128, 512], BF16)
            nc.vector.memset(wu[:], 0.0)
            wu_ps = ps.tile([128, 512], F32, tag="ps")
            # Sized to span the cold 1.2 GHz ramp and hand off to the real
            # stream (which begins as soon as gb0a+x0a land) with no idle
            # window in between, so the HAM never re-throttles.
            for _ in range(8):
                nc.tensor.matmul(wu_ps[:], wu[:, 0:128], wu[:],
                                 start=True, stop=True)
            for _ in range(4):
                nc.tensor.matmul(wu_ps[:, 0:256], wu[:, 0:128],
                                 wu[:, 0:256], start=True, stop=True)

            def gbub_dma(ci):
                s, n = CHUNKS[ci]
                nc.sync.dma_start(gbC[ci][:], gb_s[:, s * HTW:(s + n) * HTW])
                nc.sync.dma_start(ubC[ci][:], ub_s[:, s * HTW:(s + n) * HTW])

            nc.sync.dma_start(gb0a[:], gb_s[:, 0:128])
            nc.sync.dma_start(x0a[:], xg_s[:, 0:XSPL * tb0])
            nc.sync.dma_start(gb0b[:], gb_s[:, 128:HTW])
            nc.sync.dma_start(x0b[:], xg_s[:, XSPL * tb0:DC * tb0])
            nc.sync.dma_start(ubC[0][:], ub_s[:, 0:HTW])
            for ci in range(1, len(CHUNKS)):
                gbub_dma(ci)
            if nblk > 1:
                emit_xb_dma(1, nc.sync)
            nc.sync.dma_start(db_h[0][:], db_s[:, 0:NH2 * HTW])
            nc.sync.dma_start(db_h[1][:], db_s[:, NH2 * HTW:HC * HTW])
            nc.sync.dma_start(ones_sb[:], onesd[:])
            nc.sync.dma_start(wrow_sb[:], wrow[:])

            def bank_slice(bank, ht, k):
                for i, (s, n) in enumerate(CHUNKS):
                    if s <= ht < s + n:
                        return bank[i][:, ht - s, k, :]
                raise AssertionError(ht)

            def gb_slice(ht, k):
                if ht == 0:
                    return gb0a[:, :] if k == 0 else gb0b[:, k - 1, :]
                return bank_slice(gbC, ht, k)

            def ub_slice(ht, k):
                return bank_slice(ubC, ht, k)

            hts_all = {}

            def emit_g1(bi):
                # h = silu(x@gb) * (x@ub), feature-major [H, tb]
                tb = blocks[bi][1]
                hts = []
                for ht in range(HC):
                    a_ps = ps.tile([128, tb], F32, tag="ps")
                    for k in range(DC):
                        nc.tensor.matmul(a_ps[:], gb_slice(ht, k),
                                         xslice(bi, k),
                                         start=(k == 0), stop=(k == DC - 1))
                    u_ps = ps.tile([128, tb], F32, tag="ps")
                    for k in range(DC):
                        nc.tensor.matmul(u_ps[:], ub_slice(ht, k),
                                         xslice(bi, k),
                                         start=(k == 0), stop=(k == DC - 1))
                    sa = sap.tile([128, tb], F32)
                    nc.scalar.activation(sa[:], a_ps[:],
                                         mybir.ActivationFunctionType.Silu)
                    hch = hp.tile([128, tb], BF16, tag="h")
                    nc.vector.tensor_mul(hch[:], sa[:], u_ps[:])
                    hts.append(hch)
                hts_all[bi] = hts

            def emit_g2(bi):
                # y^T = db^T @ h, scaled by the per-token gate weight
                off, tb = blocks[bi]
                hts = hts_all.pop(bi)
                w_ps = ps.tile([128, tb], F32, tag="ps")
                nc.tensor.matmul(w_ps[:], ones_sb[:],
                                 wrow_sb[:, off:off + tb],
                                 start=True, stop=True)
                wsb = wsp.tile([128, tb], F32)
                nc.vector.tensor_copy(wsb[:], w_ps[:])
                last = (bi == nblk - 1)
                for dt in range(DC):
                    # The very last output chunk is split into column halves
                    # so the kernel-ending MUL+DMA chain runs on a half-size
                    # tile while the first half's chain hides under the
                    # second half's matmuls.
                    if last and dt == DC - 1:
                        hw_ = tb // 2 // 4 * 4
                        spans = [(0, hw_), (hw_, tb - hw_)]
                    else:
                        spans = [(0, tb)]
                    for c0, cw in spans:
                        y_ps = ps.tile([128, cw], F32, tag="ps")
                        for hk in range(HC):
                            half, kk = divmod(hk, NH2)
                            nc.tensor.matmul(y_ps[:], db_h[half][:, kk, dt, :],
                                             hts[hk][:, c0:c0 + cw],
                                             start=(hk == 0),
                                             stop=(hk == HC - 1))
                        ysb = yp.tile([128, cw], BF16, tag="ysb")
                        nc.vector.tensor_mul(ysb[:], y_ps[:],
                                             wsb[:, c0:c0 + cw])
                        nc.sync.dma_start(
                            y_s[:, off * DC + dt * tb + c0:
                                off * DC + dt * tb + c0 + cw], ysb[:])

            # Software-pipelined emission: the in-order PE always has the
            # next block's GEMM1 queued before this block's GEMM2, so DMA
            # pacing stalls in one never idle the other.
            emit_g1(0)
            if nblk > 1:
                emit_g1(1)
            for b in range(nblk):
                if b + 2 < nblk:
                    emit_xb_dma(b + 2, nc.sync)
                emit_g2(b)
                if b + 2 < nblk:
                    emit_g1(b + 2)

    return _split_multiwaits(_hoist_first_dmas(nc, k=4))


_NC_CACHE = {}


def _routing(x2d, gate_w):
    """Replicates the reference gate: softmax over E, top-2, renormalize."""
    logits = x2d @ gate_w.T                                  # [NTOK, E] f32
    lmax = logits.max(-1, keepdims=True)
    p = np.exp(logits - lmax)
    p = p / p.sum(-1, keepdims=True)
    idx = np.argsort(-p, axis=-1, kind="stable")[:, :KTOP]   # [NTOK, 2]
    sel = np.take_along_axis(p, idx, -1)
    w = sel / (sel.sum(-1, keepdims=True) + 1e-8)            # [NTOK, 2]
    return idx, w.astype(np.float32)


def kernel(x, gate_w, gate_bank, up_bank, down_bank, _trace=False):
    _install_axon_ntff_hook()
    x = np.asarray(x, dtype=np.float32)
    gate_w = np.asarray(gate_w, dtype=np.float32)
    x2d = np.ascontiguousarray(x.reshape(NTOK, D))

    idx, w = _routing(x2d, gate_w)

    # Token lists per expert.
    tok_idx = []
    tok_w = []
    for e in range(E):
        hit = (idx == e)                        # [NTOK, 2]
        rows = np.nonzero(hit.any(-1))[0]
        tok_idx.append(rows)
        tok_w.append(w[rows, np.argmax(hit[rows], axis=-1)])
    nmax = max(len(r) for r in tok_idx)
    npad = ((nmax + 3) // 4) * 4

    key = npad
    if key not in _NC_CACHE:
        _NC_CACHE[key] = build_nc(npad)
    nc = _NC_CACHE[key]

    blocks = _blocks(npad)
    offs = [b[0] for b in blocks]
    sizes = [b[1] for b in blocks]

    def swz_x(xgT):
        out = np.empty((128, DC * npad), NPBF16)
        for off, tb in zip(offs, sizes):
            seg = xgT[:, off:off + tb].reshape(DC, 128, tb)
            out[:, off * DC:(off + tb) * DC] = \
                seg.transpose(1, 0, 2).reshape(128, DC * tb)
        return out

    in_maps = []
    for e in range(E):
        rows = tok_idx[e]
        xg = np.zeros((npad, D), np.float32)
        xg[: len(rows)] = x2d[rows]
        wr = np.zeros((1, npad), np.float32)
        wr[0, : len(rows)] = tok_w[e]
        gbs = np.asarray(gate_bank[e], np.float32).reshape(DC, 128, HC, 128) \
            .transpose(1, 2, 0, 3).reshape(128, HC * DC * 128)
        ubs = np.asarray(up_bank[e], np.float32).reshape(DC, 128, HC, 128) \
            .transpose(1, 2, 0, 3).reshape(128, HC * DC * 128)
        dbs = np.asarray(down_bank[e], np.float32).reshape(HC, 128, DC, 128) \
            .transpose(1, 0, 2, 3).reshape(128, HC * DC * 128)
        in_maps.append({
            "xg_s": swz_x(np.ascontiguousarray(xg.T.astype(NPBF16))),
            "gb_s": np.ascontiguousarray(gbs.astype(NPBF16)),
            "ub_s": np.ascontiguousarray(ubs.astype(NPBF16)),
            "db_s": np.ascontiguousarray(dbs.astype(NPBF16)),
            "wrow": wr,
            "onesd": np.ones((1, 128), np.float32),
        })

    res = bass_utils.run_bass_kernel_spmd(
        nc, in_maps, core_ids=list(range(8)), trace=_trace)

    y = np.zeros((NTOK, D), np.float32)
    for e in range(E):
        rows = tok_idx[e]
        ys = np.asarray(res.results[e]["y_s"]).astype(np.float32)
        ygT = np.empty((D, npad), np.float32)
        for off, tb in zip(offs, sizes):
            ygT[:, off:off + tb] = ys[:, off * DC:(off + tb) * DC] \
                .reshape(128, DC, tb).transpose(1, 0, 2).reshape(D, tb)
        y[rows] += ygT[:, : len(rows)].T
    y = y.reshape(B, T, D)
    if _trace:
        return y, res
    return y


# revision 36
# speedup vs baseline: 1.0173x; 1.0173x over previous
"""MoE (B=2,T=2048,D=768,E=8,K=2,H=1536) Trainium2 kernel.

Sparse expert-parallel over the 8 NeuronCores: the host computes the gate
(softmax + top-2) in numpy, gathers the tokens routed to each expert, and
core e runs expert e's FFN only on its ~B*T*K/E gathered tokens. The
per-token gate weight is applied on device; the host scatter-adds the two
weighted expert outputs per token.

Activations stay feature-major (x^T [D, tok]) so gate/up banks [D,H] and
the down bank [H,D] are already in the stationary-operand (lhsT) layout the
PE wants — no transposes on device. The big GEMMs run in bf16 (full PE
rate, automatic fast-weight-load so LDWEIGHTS hides under the matmuls,
half the HBM traffic of f32; accumulation stays f32 in PSUM). The token
count is padded only to a multiple of 8 — tokens are the free (moving)
dim, so no 128-granularity is needed.
"""

import numpy as np
import ml_dtypes

import concourse.bass as bass
import concourse.mybir as mybir
import concourse.tile as tile
from concourse import bass_utils

# Problem shape (hardcoded per contract).
B, T, D, E, H, KTOP = 2, 2048, 768, 8, 1536, 2
NTOK = B * T            # 4096 tokens
DC = D // 128           # 6 chunks of the D (contraction) dim
HC = H // 128           # 12 chunks of the H dim
F32 = mybir.dt.float32
F32R = mybir.dt.float32r
BF16 = mybir.dt.bfloat16
NPBF16 = ml_dtypes.bfloat16
N_WARMUP = 20           # HAM warmup matmuls (8x N=512 + 12x N=256)


def _blocks(npad):
    """Token blocks, each a multiple of 8 and <=512 (PSUM bank limit).
    The first block is kept small so its x DMA lands early and the PE
    starts sooner; the rest are balanced."""
    if npad <= 512:
        return [(0, npad)]
    first = min(384, npad // 4 * 4)
    rest = npad - first
    nblk = max(1, -(-rest // 512))
    base = rest // nblk // 4 * 4
    sizes = [first] + [base] * nblk
    rem = rest - base * nblk
    i = 1
    while rem > 0:
        take = min(4, rem)
        sizes[i] += take
        rem -= take
        i = 1 + (i % nblk)
    blocks = []
    off = 0
    for s in sizes:
        blocks.append((off, s))
        off += s
    return blocks


def _install_axon_ntff_hook():
    """Best-effort: register the antenv.axon_hooks NTFF profile hook that the
    agent image lacks, so trace=True (or BASS_TRACE=1) can profile under axon.
    Never raises."""
    try:
        import sys, types, contextlib, ctypes  # noqa: PLC0415
        import antenv  # noqa: PLC0415
        if "antenv.axon_hooks" in sys.modules:
            return
        _HOOK = [None]
        mod = types.ModuleType("antenv.axon_hooks")
        mod.set_axon_ntff_profile_hook = lambda h: _HOOK.__setitem__(0, h)
        mod.get_axon_ntff_profile_hook = lambda: _HOOK[0]
        sys.modules["antenv.axon_hooks"] = mod
        antenv.axon_hooks = mod

        lib = ctypes.CDLL("/opt/axon/libaxon_pjrt.so")
        if not hasattr(lib, "axon_start_nrt_profile"):
            return
        lib.axon_start_nrt_profile.argtypes = [
            ctypes.POINTER(ctypes.c_int64), ctypes.c_size_t]
        lib.axon_start_nrt_profile.restype = ctypes.c_int64
        lib.axon_stop_nrt_profile.argtypes = [ctypes.c_char_p]
        lib.axon_stop_nrt_profile.restype = ctypes.c_int64

        @contextlib.contextmanager
        def _hook(output_dir, device_ids):
            import jax  # noqa: PLC0415
            jax.devices()
            if device_ids:
                ids = (ctypes.c_int64 * len(device_ids))(*device_ids)
                rc = lib.axon_start_nrt_profile(ids, len(device_ids))
            else:
                rc = lib.axon_start_nrt_profile(None, 0)
            if rc != 0:
                raise RuntimeError(f"axon_start_nrt_profile rc={rc}")
            try:
                yield
            finally:
                lib.axon_stop_nrt_profile(str(output_dir).encode())

        mod.set_axon_ntff_profile_hook(_hook)
    except Exception:
        pass


def _split_multiwaits(nc):
    """This walrus build only supports one sync-wait per instruction; move
    extra waits onto preceding NOPs on the same engine."""
    for fn in nc.m.functions:
        for bb in fn.blocks:
            out = []
            for ins in bb.instructions:
                si = ins.sync_info
                if si is not None and si.on_wait is not None and len(si.on_wait) > 1:
                    waits = list(si.on_wait)
                    for i, w in enumerate(waits[:-1]):
                        out.append(mybir.InstNoOp(
                            name=f"{ins.name}-sw{i}",
                            engine=ins.engine,
                            sync_info=mybir.SyncInfo(on_wait=[w], on_update=[]),
                        ))
                    si.on_wait = [waits[-1]]
                    ins.sync_info = si
                out.append(ins)
            bb.instructions = out
    return nc


def _hoist_first_dmas(nc, k=2):
    """Move the first `k` wait-free SP DMA dispatches above the TileContext
    entry barrier so their descriptors are written (and transfers run) while
    the other engines are still in the barrier. Saves ~1us of time-to-first-
    matmul; the hoisted DMAs have no waits and nothing before the barrier
    touches their target tiles."""
    fn = nc.m.functions[0]

    def hoist(engine, types, kk, contiguous=False):
        moved = []
        for bb in fn.blocks:
            for ins in bb.instructions:
                if ins.engine != engine:
                    continue
                if isinstance(ins, types):
                    si = ins.sync_info
                    if contiguous or si is None or not si.on_wait:
                        moved.append((bb, ins))
                    if len(moved) >= kk:
                        break
                elif contiguous and moved:
                    break
            if moved:
                break
        if not moved:
            return
        bb0 = fn.blocks[0]
        idx = None
        for i, ins in enumerate(bb0.instructions):
            if ins.engine == engine and isinstance(ins, mybir.InstDrain):
                idx = i
                break
        if idx is None:
            return
        for bb, ins in moved:
            bb.instructions.remove(ins)
        bb0.instructions[idx:idx] = [ins for _, ins in moved]

    hoist(mybir.EngineType.SP, mybir.InstDMACopy, k)
    # The HAM-warmup memset too: it gates the warmup matmuls, and the DMA
    # hoist above delays every engine's body start by the extra dispatch
    # time.
    hoist(mybir.EngineType.DVE, mybir.InstMemset, 1)
    # And the warmup ldweights/matmuls themselves (the contiguous run at
    # the PE body head, incl. the memset-sem wait on the first): with them
    # above the PE barrier, the PE array warms to 2.4 GHz during the
    # barrier window instead of idling in it, so the real MM stream starts
    # fully warm.
    hoist(mybir.EngineType.PE, (mybir.InstLdweights, mybir.InstMatmult),
          2 * N_WARMUP, contiguous=True)
    return nc


def build_nc(npad):
    """Expert FFN on `npad` gathered tokens (feature-major, bf16 GEMMs)."""
    blocks = _blocks(npad)
    nblk = len(blocks)

    # All inputs are host-pre-swizzled to the exact SBUF layout so every DMA
    # is fully contiguous per partition (multi-KB lines -> peak DMA BW):
    #   xg_s[p, off*DC + c*tb + t] = x^T[c*128+p, off+t]       (block-major)
    #   gb_s[p, (ht*DC + c)*128 + j] = gate_bank[c*128+p, ht*128+j]
    #   db_s[p, (hk*DC + c)*128 + j] = down_bank[hk*128+p, c*128+j]
    # Output y_s uses the same block-major layout as xg_s.
    nc = bass.Bass()
    xg_s = nc.dram_tensor("xg_s", [128, DC * npad], BF16, kind="ExternalInput")
    gb_s = nc.dram_tensor("gb_s", [128, DC * H], BF16, kind="ExternalInput")
    ub_s = nc.dram_tensor("ub_s", [128, DC * H], BF16, kind="ExternalInput")
    db_s = nc.dram_tensor("db_s", [128, HC * D], BF16, kind="ExternalInput")
    wrow = nc.dram_tensor("wrow", [1, npad], F32R, kind="ExternalInput")
    onesd = nc.dram_tensor("onesd", [1, 128], F32R, kind="ExternalInput")
    y_s = nc.dram_tensor("y_s", [128, DC * npad], BF16, kind="ExternalOutput")

    with tile.TileContext(nc) as tc:
        with (
            tc.tile_pool(name="wts", bufs=1) as wts,
            tc.tile_pool(name="xp", bufs=1) as xp,
            tc.tile_pool(name="hp", bufs=24) as hp,
            tc.tile_pool(name="sap", bufs=2) as sap,
            tc.tile_pool(name="yp", bufs=3) as yp,
            tc.tile_pool(name="wsp", bufs=2) as wsp,
            tc.tile_pool(name="ps", bufs=8, space="PSUM") as ps,
        ):
            # DMA plan. Everything rides the SP HWDGE ring (strict FIFO,
            # low latency) in EXACT consumption order, so each chunk lands
            # just ahead of the matmuls that need it and the full 358 GB/s
            # serves the critical chunk at every moment. The gate/up banks
            # stream as interleaved ht-chunks (1-ht chunks for ht0/ht1 to
            # minimize time-to-first-matmul, ht-pairs after); x / down-bank
            # / gate-weight chunks are slotted at their need times. No
            # SWDGE: its ~2us fixed latency and unpaced concurrency made
            # chunks land late (observed 2.5us PE stalls).
            HTW = DC * 128                     # swizzled width of one ht slice
            NH2 = HC // 2
            CHUNKS = [(0, 1), (1, 1), (2, 2), (4, 2), (6, 2), (8, 2), (10, 2)]
            gbC = [wts.tile([128, n, DC, 128], BF16, tag=f"gbC{i}",
                            name=f"gbC{i}") for i, (s, n) in enumerate(CHUNKS)]
            ubC = [wts.tile([128, n, DC, 128], BF16, tag=f"ubC{i}",
                            name=f"ubC{i}") for i, (s, n) in enumerate(CHUNKS)]
            db_h = [wts.tile([128, NH2, DC, 128], BF16, tag=f"dbh{i}",
                             name=f"dbh{i}") for i in range(2)]
            ones_sb = wts.tile([1, 128], F32R)
            wrow_sb = wts.tile([1, npad], F32R)

            xbs = [None] * nblk

            def emit_xb_dma(bi, ring):
                off, tb = blocks[bi]
                xbt = xp.tile([128, DC, tb], BF16,
                              tag=f"xb{'ab'[bi % 2]}", name=f"xb{bi}")
                ins = ring.dma_start(xbt[:], xg_s[:, off * DC:(off + tb) * DC])
                xbs[bi] = xbt
                return ins

            # HAM warmup: the PE clock-gate defaults to 1.2 GHz and only
            # reaches 2.4 GHz after ~3.4us of sustained activity. Feed it
            # throwaway matmuls on a memset scratch tile (no DMA dependency,
            # so they start right after the framework preamble) so the real
            # MM stream runs warm from its first instruction.
            wu = wts.tile([128, 512], BF16)
            nc.vector.memset(wu[:], 0.0)
            wu_ps = ps.tile([128, 512], F32, tag="ps")
            # 8x N=512 span the cold 1.2 GHz ramp (~3.4us); 12x N=256 keep
            # the (now warm) PE busy at fine granularity until the first
            # real operands land, so the HAM never re-throttles before the
            # real stream begins.
            for _ in range(8):
                nc.tensor.matmul(wu_ps[:], wu[:, 0:128], wu[:],
                                 start=True, stop=True)
            for _ in range(12):
                nc.tensor.matmul(wu_ps[:, 0:256], wu[:, 0:128],
                                 wu[:, 0:256], start=True, stop=True)

            def gbub_dma(ci):
                s, n = CHUNKS[ci]
                nc.sync.dma_start(gbC[ci][:], gb_s[:, s * HTW:(s + n) * HTW])
                nc.sync.dma_start(ubC[ci][:], ub_s[:, s * HTW:(s + n) * HTW])

            s, n = CHUNKS[0]
            nc.sync.dma_start(gbC[0][:], gb_s[:, s * HTW:(s + n) * HTW])
            emit_xb_dma(0, nc.sync)
            nc.sync.dma_start(ubC[0][:], ub_s[:, s * HTW:(s + n) * HTW])
            for ci in range(1, len(CHUNKS)):
                gbub_dma(ci)
            if nblk > 1:
                emit_xb_dma(1, nc.sync)
            nc.sync.dma_start(db_h[0][:], db_s[:, 0:NH2 * HTW])
            nc.sync.dma_start(db_h[1][:], db_s[:, NH2 * HTW:HC * HTW])
            nc.sync.dma_start(ones_sb[:], onesd[:])
            nc.sync.dma_start(wrow_sb[:], wrow[:])

            def bank_slice(bank, ht, k):
                for i, (s, n) in enumerate(CHUNKS):
                    if s <= ht < s + n:
                        return bank[i][:, ht - s, k, :]
                raise AssertionError(ht)

            def gb_slice(ht, k):
                return bank_slice(gbC, ht, k)

            def ub_slice(ht, k):
                return bank_slice(ubC, ht, k)

            hts_all = {}

            def emit_g1(bi):
                # h = silu(x@gb) * (x@ub), feature-major [H, tb]
                xb = xbs[bi]
                tb = blocks[bi][1]
                hts = []
                for ht in range(HC):
                    a_ps = ps.tile([128, tb], F32, tag="ps")
                    for k in range(DC):
                        nc.tensor.matmul(a_ps[:], gb_slice(ht, k),
                                         xb[:, k, :],
                                         start=(k == 0), stop=(k == DC - 1))
                    u_ps = ps.tile([128, tb], F32, tag="ps")
                    for k in range(DC):
                        nc.tensor.matmul(u_ps[:], ub_slice(ht, k),
                                         xb[:, k, :],
                                         start=(k == 0), stop=(k == DC - 1))
                    sa = sap.tile([128, tb], F32)
                    nc.scalar.activation(sa[:], a_ps[:],
                                         mybir.ActivationFunctionType.Silu)
                    hch = hp.tile([128, tb], BF16, tag="h")
                    nc.vector.tensor_mul(hch[:], sa[:], u_ps[:])
                    hts.append(hch)
                hts_all[bi] = hts

            def emit_g2(bi):
                # y^T = db^T @ h, scaled by the per-token gate weight
                off, tb = blocks[bi]
                hts = hts_all.pop(bi)
                w_ps = ps.tile([128, tb], F32, tag="ps")
                nc.tensor.matmul(w_ps[:], ones_sb[:],
                                 wrow_sb[:, off:off + tb],
                                 start=True, stop=True)
                wsb = wsp.tile([128, tb], F32)
                nc.vector.tensor_copy(wsb[:], w_ps[:])
                last = (bi == nblk - 1)
                for dt in range(DC):
                    # The very last output chunk is split into column halves
                    # so the kernel-ending MUL+DMA chain runs on a half-size
                    # tile while the first half's chain hides under the
                    # second half's matmuls.
                    if last and dt == DC - 1:
                        hw_ = tb // 2 // 4 * 4
                        spans = [(0, hw_), (hw_, tb - hw_)]
                    else:
                        spans = [(0, tb)]
                    for c0, cw in spans:
                        y_ps = ps.tile([128, cw], F32, tag="ps")
                        for hk in range(HC):
                            half, kk = divmod(hk, NH2)
                            nc.tensor.matmul(y_ps[:], db_h[half][:, kk, dt, :],
                                             hts[hk][:, c0:c0 + cw],
                                             start=(hk == 0),
                                             stop=(hk == HC - 1))
                        ysb = yp.tile([128, cw], BF16, tag="ysb")
                        nc.vector.tensor_mul(ysb[:], y_ps[:],
                                             wsb[:, c0:c0 + cw])
                        nc.sync.dma_start(
                            y_s[:, off * DC + dt * tb + c0:
                                off * DC + dt * tb + c0 + cw], ysb[:])

            # Software-pipelined emission: the in-order PE always has the
            # next block's GEMM1 queued before this block's GEMM2, so DMA
            # pacing stalls in one never idle the other.
            emit_g1(0)
            if nblk > 1:
                emit_g1(1)
            for b in range(nblk):
                if b + 2 < nblk:
                    emit_xb_dma(b + 2, nc.sync)
                emit_g2(b)
                if b + 2 < nblk:
                    emit_g1(b + 2)

    return _split_multiwaits(_hoist_first_dmas(nc, k=7))


_NC_CACHE = {}


def _routing(x2d, gate_w):
    """Replicates the reference gate: softmax over E, top-2, renormalize."""
    logits = x2d @ gate_w.T                                  # [NTOK, E] f32
    lmax = logits.max(-1, keepdims=True)
    p = np.exp(logits - lmax)
    p = p / p.sum(-1, keepdims=True)
    idx = np.argsort(-p, axis=-1, kind="stable")[:, :KTOP]   # [NTOK, 2]
    sel = np.take_along_axis(p, idx, -1)
    w = sel / (sel.sum(-1, keepdims=True) + 1e-8)            # [NTOK, 2]
    return idx, w.astype(np.float32)


def kernel(x, gate_w, gate_bank, up_bank, down_bank, _trace=False):
    _install_axon_ntff_hook()
    x = np.asarray(x, dtype=np.float32)
    gate_w = np.asarray(gate_w, dtype=np.float32)
    x2d = np.ascontiguousarray(x.reshape(NTOK, D))

    idx, w = _routing(x2d, gate_w)

    # Token lists per expert.
    tok_idx = []
    tok_w = []
    for e in range(E):
        hit = (idx == e)                        # [NTOK, 2]
        rows = np.nonzero(hit.any(-1))[0]
        tok_idx.append(rows)
        tok_w.append(w[rows, np.argmax(hit[rows], axis=-1)])
    nmax = max(len(r) for r in tok_idx)
    npad = ((nmax + 3) // 4) * 4

    key = npad
    if key not in _NC_CACHE:
        _NC_CACHE[key] = build_nc(npad)
    nc = _NC_CACHE[key]

    blocks = _blocks(npad)
    offs = [b[0] for b in blocks]
    sizes = [b[1] for b in blocks]

    def swz_x(xgT):
        out = np.empty((128, DC * npad), NPBF16)
        for off, tb in zip(offs, sizes):
            seg = xgT[:, off:off + tb].reshape(DC, 128, tb)
            out[:, off * DC:(off + tb) * DC] = \
                seg.transpose(1, 0, 2).reshape(128, DC * tb)
        return out

    in_maps = []
    for e in range(E):
        rows = tok_idx[e]
        xg = np.zeros((npad, D), np.float32)
        xg[: len(rows)] = x2d[rows]
        wr = np.zeros((1, npad), np.float32)
        wr[0, : len(rows)] = tok_w[e]
        gbs = np.asarray(gate_bank[e], np.float32).reshape(DC, 128, HC, 128) \
            .transpose(1, 2, 0, 3).reshape(128, HC * DC * 128)
        ubs = np.asarray(up_bank[e], np.float32).reshape(DC, 128, HC, 128) \
            .transpose(1, 2, 0, 3).reshape(128, HC * DC * 128)
        dbs = np.asarray(down_bank[e], np.float32).reshape(HC, 128, DC, 128) \
            .transpose(1, 0, 2, 3).reshape(128, HC * DC * 128)
        in_maps.append({
            "xg_s": swz_x(np.ascontiguousarray(xg.T.astype(NPBF16))),
            "gb_s": np.ascontiguousarray(gbs.astype(NPBF16)),
            "ub_s": np.ascontiguousarray(ubs.astype(NPBF16)),
            "db_s": np.ascontiguousarray(dbs.astype(NPBF16)),
            "wrow": wr,
            "onesd": np.ones((1, 128), np.float32),
        })

    res = bass_utils.run_bass_kernel_spmd(
        nc, in_maps, core_ids=list(range(8)), trace=_trace)

    y = np.zeros((NTOK, D), np.float32)
    for e in range(E):
        rows = tok_idx[e]
        ys = np.asarray(res.results[e]["y_s"]).astype(np.float32)
        ygT = np.empty((D, npad), np.float32)
        for off, tb in zip(offs, sizes):
            ygT[:, off:off + tb] = ys[:, off * DC:(off + tb) * DC] \
                .reshape(128, DC, tb).transpose(1, 0, 2).reshape(D, tb)
        y[rows] += ygT[:, : len(rows)].T
    y = y.reshape(B, T, D)
    if _trace:
        return y, res
    return y


# revision 41
# speedup vs baseline: 1.0218x; 1.0044x over previous
"""MoE (B=2,T=2048,D=768,E=8,K=2,H=1536) Trainium2 kernel.

Sparse expert-parallel over the 8 NeuronCores: the host computes the gate
(softmax + top-2) in numpy, gathers the tokens routed to each expert, and
core e runs expert e's FFN only on its ~B*T*K/E gathered tokens. The
per-token gate weight is applied on device; the host scatter-adds the two
weighted expert outputs per token.

Activations stay feature-major (x^T [D, tok]) so gate/up banks [D,H] and
the down bank [H,D] are already in the stationary-operand (lhsT) layout the
PE wants — no transposes on device. The big GEMMs run in bf16 (full PE
rate, automatic fast-weight-load so LDWEIGHTS hides under the matmuls,
half the HBM traffic of f32; accumulation stays f32 in PSUM). The token
count is padded only to a multiple of 8 — tokens are the free (moving)
dim, so no 128-granularity is needed.
"""

import numpy as np
import ml_dtypes

import concourse.bass as bass
import concourse.mybir as mybir
import concourse.tile as tile
from concourse import bass_utils

# Problem shape (hardcoded per contract).
B, T, D, E, H, KTOP = 2, 2048, 768, 8, 1536, 2
NTOK = B * T            # 4096 tokens
DC = D // 128           # 6 chunks of the D (contraction) dim
HC = H // 128           # 12 chunks of the H dim
F32 = mybir.dt.float32
F32R = mybir.dt.float32r
BF16 = mybir.dt.bfloat16
NPBF16 = ml_dtypes.bfloat16
N_WARMUP = 20           # HAM warmup matmuls (8x N=512 + 12x N=256)


def _blocks(npad):
    """Token blocks, each a multiple of 8 and <=512 (PSUM bank limit).
    The first block is kept small so its x DMA lands early and the PE
    starts sooner; the rest are balanced."""
    if npad <= 512:
        return [(0, npad)]
    first = min(384, npad // 4 * 4)
    rest = npad - first
    nblk = max(1, -(-rest // 512))
    base = rest // nblk // 4 * 4
    sizes = [first] + [base] * nblk
    rem = rest - base * nblk
    i = 1
    while rem > 0:
        take = min(4, rem)
        sizes[i] += take
        rem -= take
        i = 1 + (i % nblk)
    blocks = []
    off = 0
    for s in sizes:
        blocks.append((off, s))
        off += s
    return blocks


def _install_axon_ntff_hook():
    """Best-effort: register the antenv.axon_hooks NTFF profile hook that the
    agent image lacks, so trace=True (or BASS_TRACE=1) can profile under axon.
    Never raises."""
    try:
        import sys, types, contextlib, ctypes  # noqa: PLC0415
        import antenv  # noqa: PLC0415
        if "antenv.axon_hooks" in sys.modules:
            return
        _HOOK = [None]
        mod = types.ModuleType("antenv.axon_hooks")
        mod.set_axon_ntff_profile_hook = lambda h: _HOOK.__setitem__(0, h)
        mod.get_axon_ntff_profile_hook = lambda: _HOOK[0]
        sys.modules["antenv.axon_hooks"] = mod
        antenv.axon_hooks = mod

        lib = ctypes.CDLL("/opt/axon/libaxon_pjrt.so")
        if not hasattr(lib, "axon_start_nrt_profile"):
            return
        lib.axon_start_nrt_profile.argtypes = [
            ctypes.POINTER(ctypes.c_int64), ctypes.c_size_t]
        lib.axon_start_nrt_profile.restype = ctypes.c_int64
        lib.axon_stop_nrt_profile.argtypes = [ctypes.c_char_p]
        lib.axon_stop_nrt_profile.restype = ctypes.c_int64

        @contextlib.contextmanager
        def _hook(output_dir, device_ids):
            import jax  # noqa: PLC0415
            jax.devices()
            if device_ids:
                ids = (ctypes.c_int64 * len(device_ids))(*device_ids)
                rc = lib.axon_start_nrt_profile(ids, len(device_ids))
            else:
                rc = lib.axon_start_nrt_profile(None, 0)
            if rc != 0:
                raise RuntimeError(f"axon_start_nrt_profile rc={rc}")
            try:
                yield
            finally:
                lib.axon_stop_nrt_profile(str(output_dir).encode())

        mod.set_axon_ntff_profile_hook(_hook)
    except Exception:
        pass


def _split_multiwaits(nc):
    """This walrus build only supports one sync-wait per instruction; move
    extra waits onto preceding NOPs on the same engine."""
    for fn in nc.m.functions:
        for bb in fn.blocks:
            out = []
            for ins in bb.instructions:
                si = ins.sync_info
                if si is not None and si.on_wait is not None and len(si.on_wait) > 1:
                    waits = list(si.on_wait)
                    for i, w in enumerate(waits[:-1]):
                        out.append(mybir.InstNoOp(
                            name=f"{ins.name}-sw{i}",
                            engine=ins.engine,
                            sync_info=mybir.SyncInfo(on_wait=[w], on_update=[]),
                        ))
                    si.on_wait = [waits[-1]]
                    ins.sync_info = si
                out.append(ins)
            bb.instructions = out
    return nc


def _hoist_first_dmas(nc, k=2):
    """Move the first `k` wait-free SP DMA dispatches above the TileContext
    entry barrier so their descriptors are written (and transfers run) while
    the other engines are still in the barrier. Saves ~1us of time-to-first-
    matmul; the hoisted DMAs have no waits and nothing before the barrier
    touches their target tiles."""
    fn = nc.m.functions[0]

    def hoist(engine, types, kk, contiguous=False):
        moved = []
        for bb in fn.blocks:
            for ins in bb.instructions:
                if ins.engine != engine:
                    continue
                if isinstance(ins, types):
                    si = ins.sync_info
                    if contiguous or si is None or not si.on_wait:
                        moved.append((bb, ins))
                    if len(moved) >= kk:
                        break
                elif contiguous and moved:
                    break
            if moved:
                break
        if not moved:
            return
        bb0 = fn.blocks[0]
        idx = None
        for i, ins in enumerate(bb0.instructions):
            if ins.engine == engine and isinstance(ins, mybir.InstDrain):
                idx = i
                break
        if idx is None:
            return
        for bb, ins in moved:
            bb.instructions.remove(ins)
        bb0.instructions[idx:idx] = [ins for _, ins in moved]

    hoist(mybir.EngineType.SP, mybir.InstDMACopy, k)
    # The HAM-warmup memset too: it gates the warmup matmuls, and the DMA
    # hoist above delays every engine's body start by the extra dispatch
    # time.
    hoist(mybir.EngineType.DVE, mybir.InstMemset, 1)
    # And the warmup ldweights/matmuls themselves (the contiguous run at
    # the PE body head, incl. the memset-sem wait on the first): with them
    # above the PE barrier, the PE array warms to 2.4 GHz during the
    # barrier window instead of idling in it, so the real MM stream starts
    # fully warm.
    hoist(mybir.EngineType.PE, (mybir.InstLdweights, mybir.InstMatmult),
          2 * N_WARMUP, contiguous=True)
    return nc


def build_nc(npad):
    """Expert FFN on `npad` gathered tokens (feature-major, bf16 GEMMs)."""
    blocks = _blocks(npad)
    nblk = len(blocks)

    # All inputs are host-pre-swizzled to the exact SBUF layout so every DMA
    # is fully contiguous per partition (multi-KB lines -> peak DMA BW):
    #   xg_s[p, off*DC + c*tb + t] = x^T[c*128+p, off+t]       (block-major)
    #   gb_s[p, (ht*DC + c)*128 + j] = gate_bank[c*128+p, ht*128+j]
    #   db_s[p, (hk*DC + c)*128 + j] = down_bank[hk*128+p, c*128+j]
    # Output y_s uses the same block-major layout as xg_s.
    nc = bass.Bass()
    xg_s = nc.dram_tensor("xg_s", [128, DC * npad], BF16, kind="ExternalInput")
    gb_s = nc.dram_tensor("gb_s", [128, DC * H], BF16, kind="ExternalInput")
    ub_s = nc.dram_tensor("ub_s", [128, DC * H], BF16, kind="ExternalInput")
    db_s = nc.dram_tensor("db_s", [128, HC * D], BF16, kind="ExternalInput")
    # Per-token gate weights pre-broadcast to all 128 partitions by the
    # host: a plain DMA replaces the f32r ones-outer-product matmul, whose
    # PE dtype-mode switch stalled the bf16 stream at every GEMM2 start.
    wsb_s = nc.dram_tensor("wsb_s", [128, npad], F32, kind="ExternalInput")
    y_s = nc.dram_tensor("y_s", [128, DC * npad], BF16, kind="ExternalOutput")

    with tile.TileContext(nc) as tc:
        with (
            tc.tile_pool(name="wts", bufs=1) as wts,
            tc.tile_pool(name="xp", bufs=1) as xp,
            tc.tile_pool(name="hp", bufs=24) as hp,
            tc.tile_pool(name="sap", bufs=2) as sap,
            tc.tile_pool(name="yp", bufs=3) as yp,
            tc.tile_pool(name="wsp", bufs=2) as wsp,
            tc.tile_pool(name="ps", bufs=8, space="PSUM") as ps,
        ):
            # DMA plan. Everything rides the SP HWDGE ring (strict FIFO,
            # low latency) in EXACT consumption order, so each chunk lands
            # just ahead of the matmuls that need it and the full 358 GB/s
            # serves the critical chunk at every moment. The gate/up banks
            # stream as interleaved ht-chunks (1-ht chunks for ht0/ht1 to
            # minimize time-to-first-matmul, ht-pairs after); x / down-bank
            # / gate-weight chunks are slotted at their need times. No
            # SWDGE: its ~2us fixed latency and unpaced concurrency made
            # chunks land late (observed 2.5us PE stalls).
            HTW = DC * 128                     # swizzled width of one ht slice
            NH2 = HC // 2
            CHUNKS = [(0, 1), (1, 1), (2, 2), (4, 2), (6, 2), (8, 2), (10, 2)]
            gbC = [wts.tile([128, n, DC, 128], BF16, tag=f"gbC{i}",
                            name=f"gbC{i}") for i, (s, n) in enumerate(CHUNKS)]
            ubC = [wts.tile([128, n, DC, 128], BF16, tag=f"ubC{i}",
                            name=f"ubC{i}") for i, (s, n) in enumerate(CHUNKS)]
            db_h = [wts.tile([128, NH2, DC, 128], BF16, tag=f"dbh{i}",
                             name=f"dbh{i}") for i in range(2)]
            wsbs = [wts.tile([128, blocks[b][1]], F32, tag=f"wsb{b}",
                             name=f"wsb{b}") for b in range(nblk)]

            xbs = [None] * nblk

            def emit_xb_dma(bi, ring):
                off, tb = blocks[bi]
                xbt = xp.tile([128, DC, tb], BF16,
                              tag=f"xb{'ab'[bi % 2]}", name=f"xb{bi}")
                ins = ring.dma_start(xbt[:], xg_s[:, off * DC:(off + tb) * DC])
                xbs[bi] = xbt
                return ins

            # HAM warmup: the PE clock-gate defaults to 1.2 GHz and only
            # reaches 2.4 GHz after ~3.4us of sustained activity. Feed it
            # throwaway matmuls on a memset scratch tile (no DMA dependency,
            # so they start right after the framework preamble) so the real
            # MM stream runs warm from its first instruction.
            wu = wts.tile([128, 512], BF16)
            nc.vector.memset(wu[:], 0.0)
            wu_ps = ps.tile([128, 512], F32, tag="ps")
            # 8x N=512 span the cold 1.2 GHz ramp (~3.4us); 12x N=256 keep
            # the (now warm) PE busy at fine granularity until the first
            # real operands land, so the HAM never re-throttles before the
            # real stream begins.
            for _ in range(8):
                nc.tensor.matmul(wu_ps[:], wu[:, 0:128], wu[:],
                                 start=True, stop=True)
            for _ in range(12):
                nc.tensor.matmul(wu_ps[:, 0:256], wu[:, 0:128],
                                 wu[:, 0:256], start=True, stop=True)

            def gbub_dma(ci):
                s, n = CHUNKS[ci]
                nc.sync.dma_start(gbC[ci][:], gb_s[:, s * HTW:(s + n) * HTW])
                nc.sync.dma_start(ubC[ci][:], ub_s[:, s * HTW:(s + n) * HTW])

            s, n = CHUNKS[0]
            nc.sync.dma_start(gbC[0][:], gb_s[:, s * HTW:(s + n) * HTW])
            emit_xb_dma(0, nc.sync)
            nc.sync.dma_start(ubC[0][:], ub_s[:, s * HTW:(s + n) * HTW])
            for ci in range(1, len(CHUNKS)):
                gbub_dma(ci)
            if nblk > 1:
                emit_xb_dma(1, nc.sync)
            nc.sync.dma_start(db_h[0][:], db_s[:, 0:NH2 * HTW])
            nc.sync.dma_start(db_h[1][:], db_s[:, NH2 * HTW:HC * HTW])
            for b in range(nblk):
                off, tb = blocks[b]
                nc.sync.dma_start(wsbs[b][:], wsb_s[:, off:off + tb])

            def bank_slice(bank, ht, k):
                for i, (s, n) in enumerate(CHUNKS):
                    if s <= ht < s + n:
                        return bank[i][:, ht - s, k, :]
                raise AssertionError(ht)

            def gb_slice(ht, k):
                return bank_slice(gbC, ht, k)

            def ub_slice(ht, k):
                return bank_slice(ubC, ht, k)

            hts_all = {}

            def emit_g1(bi):
                # h = silu(x@gb) * (x@ub), feature-major [H, tb]
                xb = xbs[bi]
                tb = blocks[bi][1]
                hts = []
                for ht in range(HC):
                    a_ps = ps.tile([128, tb], F32, tag="ps")
                    for k in range(DC):
                        nc.tensor.matmul(a_ps[:], gb_slice(ht, k),
                                         xb[:, k, :],
                                         start=(k == 0), stop=(k == DC - 1))
                    u_ps = ps.tile([128, tb], F32, tag="ps")
                    for k in range(DC):
                        nc.tensor.matmul(u_ps[:], ub_slice(ht, k),
                                         xb[:, k, :],
                                         start=(k == 0), stop=(k == DC - 1))
                    sa = sap.tile([128, tb], F32)
                    nc.scalar.activation(sa[:], a_ps[:],
                                         mybir.ActivationFunctionType.Silu)
                    hch = hp.tile([128, tb], BF16, tag="h")
                    nc.vector.tensor_mul(hch[:], sa[:], u_ps[:])
                    hts.append(hch)
                hts_all[bi] = hts

            def emit_g2(bi):
                # y^T = db^T @ h, scaled by the per-token gate weight
                off, tb = blocks[bi]
                hts = hts_all.pop(bi)
                wsb = wsbs[bi]
                last = (bi == nblk - 1)
                for dt in range(DC):
                    # The very last output chunk is split into column halves
                    # so the kernel-ending MUL+DMA chain runs on a half-size
                    # tile while the first half's chain hides under the
                    # second half's matmuls.
                    if last and dt == DC - 1:
                        hw_ = tb // 2 // 4 * 4
                        spans = [(0, hw_), (hw_, tb - hw_)]
                    else:
                        spans = [(0, tb)]
                    for c0, cw in spans:
                        y_ps = ps.tile([128, cw], F32, tag="ps")
                        for hk in range(HC):
                            half, kk = divmod(hk, NH2)
                            nc.tensor.matmul(y_ps[:], db_h[half][:, kk, dt, :],
                                             hts[hk][:, c0:c0 + cw],
                                             start=(hk == 0),
                                             stop=(hk == HC - 1))
                        ysb = yp.tile([128, cw], BF16, tag="ysb")
                        nc.vector.tensor_mul(ysb[:], y_ps[:],
                                             wsb[:, c0:c0 + cw])
                        nc.sync.dma_start(
                            y_s[:, off * DC + dt * tb + c0:
                                off * DC + dt * tb + c0 + cw], ysb[:])

            # Software-pipelined emission: the in-order PE always has the
            # next block's GEMM1 queued before this block's GEMM2, so DMA
            # pacing stalls in one never idle the other.
            emit_g1(0)
            if nblk > 1:
                emit_g1(1)
            for b in range(nblk):
                if b + 2 < nblk:
                    emit_xb_dma(b + 2, nc.sync)
                emit_g2(b)
                if b + 2 < nblk:
                    emit_g1(b + 2)

    return _split_multiwaits(_hoist_first_dmas(nc, k=7))


_NC_CACHE = {}


def _routing(x2d, gate_w):
    """Replicates the reference gate: softmax over E, top-2, renormalize."""
    logits = x2d @ gate_w.T                                  # [NTOK, E] f32
    lmax = logits.max(-1, keepdims=True)
    p = np.exp(logits - lmax)
    p = p / p.sum(-1, keepdims=True)
    idx = np.argsort(-p, axis=-1, kind="stable")[:, :KTOP]   # [NTOK, 2]
    sel = np.take_along_axis(p, idx, -1)
    w = sel / (sel.sum(-1, keepdims=True) + 1e-8)            # [NTOK, 2]
    return idx, w.astype(np.float32)


def kernel(x, gate_w, gate_bank, up_bank, down_bank, _trace=False):
    _install_axon_ntff_hook()
    x = np.asarray(x, dtype=np.float32)
    gate_w = np.asarray(gate_w, dtype=np.float32)
    x2d = np.ascontiguousarray(x.reshape(NTOK, D))

    idx, w = _routing(x2d, gate_w)

    # Token lists per expert.
    tok_idx = []
    tok_w = []
    for e in range(E):
        hit = (idx == e)                        # [NTOK, 2]
        rows = np.nonzero(hit.any(-1))[0]
        tok_idx.append(rows)
        tok_w.append(w[rows, np.argmax(hit[rows], axis=-1)])
    nmax = max(len(r) for r in tok_idx)
    npad = ((nmax + 3) // 4) * 4

    key = npad
    if key not in _NC_CACHE:
        _NC_CACHE[key] = build_nc(npad)
    nc = _NC_CACHE[key]

    blocks = _blocks(npad)
    offs = [b[0] for b in blocks]
    sizes = [b[1] for b in blocks]

    def swz_x(xgT):
        out = np.empty((128, DC * npad), NPBF16)
        for off, tb in zip(offs, sizes):
            seg = xgT[:, off:off + tb].reshape(DC, 128, tb)
            out[:, off * DC:(off + tb) * DC] = \
                seg.transpose(1, 0, 2).reshape(128, DC * tb)
        return out

    in_maps = []
    for e in range(E):
        rows = tok_idx[e]
        xg = np.zeros((npad, D), np.float32)
        xg[: len(rows)] = x2d[rows]
        wr = np.zeros((1, npad), np.float32)
        wr[0, : len(rows)] = tok_w[e]
        gbs = np.asarray(gate_bank[e], np.float32).reshape(DC, 128, HC, 128) \
            .transpose(1, 2, 0, 3).reshape(128, HC * DC * 128)
        ubs = np.asarray(up_bank[e], np.float32).reshape(DC, 128, HC, 128) \
            .transpose(1, 2, 0, 3).reshape(128, HC * DC * 128)
        dbs = np.asarray(down_bank[e], np.float32).reshape(HC, 128, DC, 128) \
            .transpose(1, 0, 2, 3).reshape(128, HC * DC * 128)
        in_maps.append({
            "xg_s": swz_x(np.ascontiguousarray(xg.T.astype(NPBF16))),
            "gb_s": np.ascontiguousarray(gbs.astype(NPBF16)),
            "ub_s": np.ascontiguousarray(ubs.astype(NPBF16)),
            "db_s": np.ascontiguousarray(dbs.astype(NPBF16)),
            "wsb_s": np.ascontiguousarray(
                np.broadcast_to(wr, (128, npad))),
        })

    res = bass_utils.run_bass_kernel_spmd(
        nc, in_maps, core_ids=list(range(8)), trace=_trace)

    y = np.zeros((NTOK, D), np.float32)
    for e in range(E):
        rows = tok_idx[e]
        ys = np.asarray(res.results[e]["y_s"]).astype(np.float32)
        ygT = np.empty((D, npad), np.float32)
        for off, tb in zip(offs, sizes):
            ygT[:, off:off + tb] = ys[:, off * DC:(off + tb) * DC] \
                .reshape(128, DC, tb).transpose(1, 0, 2).reshape(D, tb)
        y[rows] += ygT[:, : len(rows)].T
    y = y.reshape(B, T, D)
    if _trace:
        return y, res
    return y
